# revision 17
# baseline (speedup 1.0000x reference)
"""DynamicMultiLinear (MoE-style grouped linear) Trainium2 kernel.

Problem: y[i] = x[i] @ W[g(i)].T + b[g(i)] where rows of x are contiguous
segments per network g (G=256 networks, IN=OUT=256, N=262144 rows).

Strategy (expert-parallel, per sharding hint):
  - Shard the group axis over 8 cores: core d owns networks [32d, 32d+32)
    and their contiguous row segments of x.
  - Host packs, per core, a transposed + 128-padded activation tensor
    xp[j, p, k, r] = x_seg[r, k*128+p] so the device only does dense,
    statically-shaped work (same program on all 8 cores; pad count B =
    ceil(max(counts)/128) blocks per network is a compile-time constant).
  - Device: for each network j, yT[m-chunk] = sum_k W^T[k,m-chunk].T @ xT[k]
    via float32r matmuls (full fp32 precision, 1 cycle/row at N>=256),
    bias fused into the PSUM->SBUF copy on the scalar engine.
  - Host unpads/transposes back to y[N, 256].
"""

import sys
from contextlib import ExitStack

import numpy as np

if "/opt/trn_rl_repo" not in sys.path:
    sys.path.insert(0, "/opt/trn_rl_repo")

G = 256
IN_F = 256
OUT_F = 256
N_CORES = 8
GPC = G // N_CORES  # networks per core
MODE = "fp32"  # "fp32" (exact) or "bf16" (faster, ~1e-3 error)


def _row_groups(B):
    """Split B 128-row blocks into matmul row-group sizes <= 512 (fp32 moving
    operand limit), as few groups as possible."""
    groups = []
    rem = B
    while rem > 0:
        take = min(rem, 4)
        groups.append(take * 128)
        rem -= take
    return groups


def _split_multi_waits(bir):
    """The walrus build here supports only ONE sem-wait per instruction
    (setupSyncWait: 'Too many sync wait commands'). Hoist extra waits onto
    same-engine NoOps inserted directly before the instruction — engines
    dispatch their stream in order, so gating semantics are preserved."""
    changed = False
    for fn in bir["functions"]:
        for bb in fn["blocks"]:
            out = []
            for ins in bb["instructions"]:
                si = ins.get("sync_info")
                waits = (si or {}).get("on_wait") or []
                if len(waits) > 1:
                    changed = True
                    for i, w in enumerate(waits[:-1]):
                        out.append(
                            {
                                "debug": ins.get("debug", 0),
                                "engine": ins["engine"],
                                "ins": [],
                                "name": f"{ins['name']}-sw{i}",
                                "opcode": "NoOp",
                                "outs": [],
                                "sync_info": {"on_update": [], "on_wait": [w]},
                                "text_hint": "split_wait",
                            }
                        )
                    si["on_wait"] = [waits[-1]]
                out.append(ins)
            bb["instructions"] = out
    return changed


def _patch_to_json():
    import json

    import concourse.bass as bass

    if getattr(bass.Bass, "_split_waits_patched", False):
        return
    orig = bass.Bass.to_json_bytes

    def to_json_bytes(self):
        data = orig(self)
        bir = json.loads(data)
        if _split_multi_waits(bir):
            data = json.dumps(bir).encode()
        return data

    bass.Bass.to_json_bytes = to_json_bytes
    bass.Bass._split_waits_patched = True


def _patch_tile_drain():
    """The walrus build in this container rejects sem waits on InstDrain
    ("Too many sync wait commands", CoreV3GenImpl setupSyncWait). Re-emit the
    TileContext exit drain's waits as NOPs (which do accept waits) ahead of a
    wait-free drain — same sequencer, same semantics."""
    import bass_rust
    from concourse import tile
    from concourse.tile import ScopedClock

    if getattr(tile.TileContext, "_drain_patched", False):
        return

    def _drain_and_barrier(self, tick_clock, wait_clock):
        gc = tick_clock.global_clock
        procs = [i for i in range(27) if gc[i] > 0]
        maxw = 1
        for i0 in range(0, len(procs), maxw):
            nop = self.nc.sync.nop(nofuse=True, hint="predrain_wait")
            masked = bass_rust.VectorClock()
            for i in procs[i0 : i0 + maxw]:
                masked.require_at_least(i, gc[i])
            wait_clock.add_sem_waits(nop.ins, ScopedClock({None: masked}))
        self.nc.sync.drain()
        self.nc.all_engine_barrier()
        popped = self.nc._tile_sem_poison_stack.pop()
        assert popped is self._sem_poison
        self.nc.clear_and_free_semaphores(list(self.sems.allocated().values()))
        self.nc.all_engine_barrier()

    tile.TileContext._drain_and_barrier = _drain_and_barrier
    tile.TileContext._drain_patched = True


def _build_program(B, gpc=GPC, mode=None):
    import concourse.bass as bass
    import concourse.mybir as mybir
    from concourse import tile

    _patch_tile_drain()
    _patch_to_json()

    mode = mode or MODE
    R = B * 128
    f32 = mybir.dt.float32
    mdt = mybir.dt.bfloat16 if mode == "bf16" else f32

    nc = bass.Bass(target_bir_lowering=False)
    xp = nc.declare_dram_parameter("xp", [gpc, 128, 2 * R], mdt, isOutput=False)
    wt = nc.declare_dram_parameter("wt", [128, gpc * 512], mdt, isOutput=False)
    bt = nc.declare_dram_parameter("bt", [128, gpc * 2], f32, isOutput=False)
    yp = nc.declare_dram_parameter("yp", [gpc, 128, 2 * R], f32, isOutput=True)

    rgs = _row_groups(B)
    with ExitStack() as ctx:
        tc = ctx.enter_context(tile.TileContext(nc))
        wpool = ctx.enter_context(tc.tile_pool(name="w", bufs=1))
        bpool = ctx.enter_context(tc.tile_pool(name="b", bufs=1))
        xpool = ctx.enter_context(tc.tile_pool(name="x", bufs=6))
        ypool = ctx.enter_context(tc.tile_pool(name="y", bufs=4))
        pspool = ctx.enter_context(tc.tile_pool(name="ps", bufs=8, space="PSUM"))

        # weights preloaded in ~1MB quads so the first matmuls aren't gated on
        # a single huge preload DMA
        quad = 4
        while gpc % quad:
            quad //= 2
        w_quads = []
        for q in range(gpc // quad):
            w_q = wpool.tile([128, quad * 512], mdt, tag=f"w{q}")
            nc.sync.dma_start(w_q[:], wt[:, q * quad * 512 : (q + 1) * quad * 512])
            w_quads.append(w_q)
        w_tiles = [
            w_quads[j // quad][:, (j % quad) * 512 : (j % quad + 1) * 512]
            for j in range(gpc)
        ]
        b_all = bpool.tile([128, gpc * 2], f32)
        nc.sync.dma_start(b_all[:], bt[:])

        for j in range(gpc):
            x_t = xpool.tile([128, 2 * R], mdt)
            nc.sync.dma_start(x_t[:], xp[j])
            y_t = ypool.tile([128, 2 * R], f32)
            for m in range(2):
                pss = [
                    pspool.tile([128, rg], f32, tag="ps", name="ps") for rg in rgs
                ]
                # k outer, rg inner: consecutive matmuls share the stationary
                # operand (weight chunk) so PE weight reloads pipeline
                for k in range(2):
                    w0 = k * 256 + m * 128
                    r0 = 0
                    for ps, rg in zip(pss, rgs):
                        nc.tensor.matmul(
                            ps[:],
                            w_tiles[j][:, w0 : w0 + 128],
                            x_t[:, k * R + r0 : k * R + r0 + rg],
                            start=(k == 0),
                            stop=(k == 1),
                        )
                        r0 += rg
                r0 = 0
                for ps, rg in zip(pss, rgs):
                    nc.scalar.activation(
                        y_t[:, m * R + r0 : m * R + r0 + rg],
                        ps[:],
                        mybir.ActivationFunctionType.Identity,
                        bias=b_all[:, 2 * j + m : 2 * j + m + 1],
                    )
                    r0 += rg
            nc.sync.dma_start(yp[j], y_t[:])
    return nc


_PROGRAM_CACHE = {}


def _get_program(B):
    key = (B, MODE)
    if key not in _PROGRAM_CACHE:
        _PROGRAM_CACHE[key] = _build_program(B)
    return _PROGRAM_CACHE[key]


def _pack_inputs(weight, bias, x, counts, offs, B):
    """Build the 8 per-core input maps (transpose + pad on host)."""
    if MODE == "bf16":
        import ml_dtypes

        mdt = np.dtype(ml_dtypes.bfloat16)
    else:
        mdt = np.dtype(np.float32)
    R = B * 128
    in_maps = []
    for d in range(N_CORES):
        xp = np.zeros((GPC, 128, 2, R), mdt)
        for j in range(GPC):
            g = d * GPC + j
            c = int(counts[g])
            if c:
                seg = x[offs[g] : offs[g] + c]  # [c, 256]
                xp[j, :, :, :c] = seg.T.reshape(2, 128, c).transpose(1, 0, 2)
        W = weight[d * GPC : (d + 1) * GPC]  # [GPC, OUT, IN]
        wt = W.reshape(GPC, OUT_F, 2, 128).transpose(3, 0, 2, 1)  # [128,GPC,2,OUT]
        bt = bias[d * GPC : (d + 1) * GPC, 0, :].reshape(GPC, 2, 128)
        bt = bt.transpose(2, 0, 1)  # [128, GPC, 2]
        in_maps.append(
            {
                "xp": np.ascontiguousarray(xp.reshape(GPC, 128, 2 * R)),
                "wt": np.ascontiguousarray(wt.reshape(128, GPC * 512)).astype(mdt),
                "bt": np.ascontiguousarray(bt.reshape(128, GPC * 2)),
            }
        )
    return in_maps


def _unpack_outputs(results, counts, offs, B, n):
    R = B * 128
    y = np.empty((n, OUT_F), np.float32)
    for d in range(N_CORES):
        ypd = np.asarray(results[d]["yp"]).reshape(GPC, 128, 2, R)
        for j in range(GPC):
            g = d * GPC + j
            c = int(counts[g])
            if c:
                y[offs[g] : offs[g] + c] = (
                    ypd[j, :, :, :c].transpose(1, 0, 2).reshape(256, c).T
                )
    return y


def kernel(weight, bias, x, counts):
    from concourse.bass_utils import run_bass_kernel_spmd

    weight = np.ascontiguousarray(np.asarray(weight), dtype=np.float32)
    bias = np.ascontiguousarray(np.asarray(bias), dtype=np.float32)
    x = np.ascontiguousarray(np.asarray(x), dtype=np.float32)
    counts = np.asarray(counts).astype(np.int64)
    n = x.shape[0]
    offs = np.zeros(G + 1, np.int64)
    np.cumsum(counts, out=offs[1:])
    B = max(1, -(-int(counts.max()) // 128))

    nc = _get_program(B)
    in_maps = _pack_inputs(weight, bias, x, counts, offs, B)
    res = run_bass_kernel_spmd(nc, in_maps, list(range(N_CORES)))
    return _unpack_outputs(res.results, counts, offs, B, n)


# revision 18
# speedup vs baseline: 1.0551x; 1.0551x over previous
"""DynamicMultiLinear (MoE-style grouped linear) Trainium2 kernel.

Problem: y[i] = x[i] @ W[g(i)].T + b[g(i)] where rows of x are contiguous
segments per network g (G=256 networks, IN=OUT=256, N=262144 rows).

Strategy (expert-parallel, per sharding hint):
  - Shard the group axis over 8 cores: core d owns networks [32d, 32d+32)
    and their contiguous row segments of x.
  - Host packs, per core, a transposed + 128-padded activation tensor
    xp[j, p, k, r] = x_seg[r, k*128+p] so the device only does dense,
    statically-shaped work (same program on all 8 cores; pad count B =
    ceil(max(counts)/128) blocks per network is a compile-time constant).
  - Device: for each network j, yT[m-chunk] = sum_k W^T[k,m-chunk].T @ xT[k]
    via float32r matmuls (full fp32 precision, 1 cycle/row at N>=256),
    bias fused into the PSUM->SBUF copy on the scalar engine.
  - Host unpads/transposes back to y[N, 256].
"""

import sys
from contextlib import ExitStack

import numpy as np

if "/opt/trn_rl_repo" not in sys.path:
    sys.path.insert(0, "/opt/trn_rl_repo")

G = 256
IN_F = 256
OUT_F = 256
N_CORES = 8
GPC = G // N_CORES  # networks per core
MODE = "fp32"  # "fp32" (exact) or "bf16" (faster, ~1e-3 error)


def _row_groups(B):
    """Split B 128-row blocks into matmul row-group sizes <= 512 (fp32 moving
    operand limit), as few groups as possible."""
    groups = []
    rem = B
    while rem > 0:
        take = min(rem, 4)
        groups.append(take * 128)
        rem -= take
    return groups


def _split_multi_waits(bir):
    """The walrus build here supports only ONE sem-wait per instruction
    (setupSyncWait: 'Too many sync wait commands'). Hoist extra waits onto
    same-engine NoOps inserted directly before the instruction — engines
    dispatch their stream in order, so gating semantics are preserved."""
    changed = False
    for fn in bir["functions"]:
        for bb in fn["blocks"]:
            out = []
            for ins in bb["instructions"]:
                si = ins.get("sync_info")
                waits = (si or {}).get("on_wait") or []
                if len(waits) > 1:
                    changed = True
                    for i, w in enumerate(waits[:-1]):
                        out.append(
                            {
                                "debug": ins.get("debug", 0),
                                "engine": ins["engine"],
                                "ins": [],
                                "name": f"{ins['name']}-sw{i}",
                                "opcode": "NoOp",
                                "outs": [],
                                "sync_info": {"on_update": [], "on_wait": [w]},
                                "text_hint": "split_wait",
                            }
                        )
                    si["on_wait"] = [waits[-1]]
                out.append(ins)
            bb["instructions"] = out
    return changed


def _patch_to_json():
    import json

    import concourse.bass as bass

    if getattr(bass.Bass, "_split_waits_patched", False):
        return
    orig = bass.Bass.to_json_bytes

    def to_json_bytes(self):
        data = orig(self)
        bir = json.loads(data)
        if _split_multi_waits(bir):
            data = json.dumps(bir).encode()
        return data

    bass.Bass.to_json_bytes = to_json_bytes
    bass.Bass._split_waits_patched = True


def _patch_tile_drain():
    """The walrus build in this container rejects sem waits on InstDrain
    ("Too many sync wait commands", CoreV3GenImpl setupSyncWait). Re-emit the
    TileContext exit drain's waits as NOPs (which do accept waits) ahead of a
    wait-free drain — same sequencer, same semantics."""
    import bass_rust
    from concourse import tile
    from concourse.tile import ScopedClock

    if getattr(tile.TileContext, "_drain_patched", False):
        return

    def _drain_and_barrier(self, tick_clock, wait_clock):
        gc = tick_clock.global_clock
        procs = [i for i in range(27) if gc[i] > 0]
        maxw = 1
        for i0 in range(0, len(procs), maxw):
            nop = self.nc.sync.nop(nofuse=True, hint="predrain_wait")
            masked = bass_rust.VectorClock()
            for i in procs[i0 : i0 + maxw]:
                masked.require_at_least(i, gc[i])
            wait_clock.add_sem_waits(nop.ins, ScopedClock({None: masked}))
        self.nc.sync.drain()
        self.nc.all_engine_barrier()
        popped = self.nc._tile_sem_poison_stack.pop()
        assert popped is self._sem_poison
        self.nc.clear_and_free_semaphores(list(self.sems.allocated().values()))
        self.nc.all_engine_barrier()

    tile.TileContext._drain_and_barrier = _drain_and_barrier
    tile.TileContext._drain_patched = True


def _build_program(B, gpc=GPC, mode=None):
    import concourse.bass as bass
    import concourse.mybir as mybir
    from concourse import tile

    _patch_tile_drain()
    _patch_to_json()

    mode = mode or MODE
    R = B * 128
    f32 = mybir.dt.float32
    mdt = mybir.dt.bfloat16 if mode == "bf16" else f32

    nc = bass.Bass(target_bir_lowering=False)
    xp = nc.declare_dram_parameter("xp", [gpc, 128, 2 * R], mdt, isOutput=False)
    wt = nc.declare_dram_parameter("wt", [128, gpc * 512], mdt, isOutput=False)
    bt = nc.declare_dram_parameter("bt", [128, gpc * 2], f32, isOutput=False)
    yp = nc.declare_dram_parameter("yp", [gpc, 128, 2 * R], f32, isOutput=True)

    rgs = _row_groups(B)
    with ExitStack() as ctx:
        tc = ctx.enter_context(tile.TileContext(nc))
        wpool = ctx.enter_context(tc.tile_pool(name="w", bufs=1))
        bpool = ctx.enter_context(tc.tile_pool(name="b", bufs=1))
        xpool = ctx.enter_context(tc.tile_pool(name="x", bufs=6))
        ypool = ctx.enter_context(tc.tile_pool(name="y", bufs=4))
        pspool = ctx.enter_context(tc.tile_pool(name="ps", bufs=8, space="PSUM"))

        # weight quads (~1MB DMAs) interleaved into the network loop so the
        # first matmuls are gated only on the first quad + first x tile
        quad = 4
        while gpc % quad:
            quad //= 2
        w_quads = [None] * (gpc // quad)

        def load_quad(q):
            w_q = wpool.tile([128, quad * 512], mdt, tag=f"w{q}", name=f"w{q}")
            nc.sync.dma_start(w_q[:], wt[:, q * quad * 512 : (q + 1) * quad * 512])
            w_quads[q] = w_q

        b_all = bpool.tile([128, gpc * 2], f32)

        yp4 = yp.rearrange("j p (m r) -> j p m r", m=2)
        for j in range(gpc):
            x_t = xpool.tile([128, 2 * R], mdt)
            nc.sync.dma_start(x_t[:], xp[j])
            if j % quad == 0:
                load_quad(j // quad)
            if j == 0:
                nc.sync.dma_start(b_all[:], bt[:])
            w_j = w_quads[j // quad][:, (j % quad) * 512 : (j % quad + 1) * 512]
            y_t = ypool.tile([128, 2 * R], f32)
            for m in range(2):
                pss = [
                    pspool.tile([128, rg], f32, tag="ps", name="ps") for rg in rgs
                ]
                # k outer, rg inner: consecutive matmuls share the stationary
                # operand (weight chunk) so PE weight reloads pipeline
                for k in range(2):
                    w0 = k * 256 + m * 128
                    r0 = 0
                    for ps, rg in zip(pss, rgs):
                        nc.tensor.matmul(
                            ps[:],
                            w_j[:, w0 : w0 + 128],
                            x_t[:, k * R + r0 : k * R + r0 + rg],
                            start=(k == 0),
                            stop=(k == 1),
                        )
                        r0 += rg
                r0 = 0
                for ps, rg in zip(pss, rgs):
                    nc.scalar.activation(
                        y_t[:, m * R + r0 : m * R + r0 + rg],
                        ps[:],
                        mybir.ActivationFunctionType.Identity,
                        bias=b_all[:, 2 * j + m : 2 * j + m + 1],
                    )
                    r0 += rg
                # per-m output DMA so the kernel tail is one half-tile deep
                nc.sync.dma_start(yp4[j, :, m, :], y_t[:, m * R : (m + 1) * R])
    return nc


_PROGRAM_CACHE = {}


def _get_program(B):
    key = (B, MODE)
    if key not in _PROGRAM_CACHE:
        _PROGRAM_CACHE[key] = _build_program(B)
    return _PROGRAM_CACHE[key]


def _pack_inputs(weight, bias, x, counts, offs, B):
    """Build the 8 per-core input maps (transpose + pad on host)."""
    if MODE == "bf16":
        import ml_dtypes

        mdt = np.dtype(ml_dtypes.bfloat16)
    else:
        mdt = np.dtype(np.float32)
    R = B * 128
    in_maps = []
    for d in range(N_CORES):
        xp = np.zeros((GPC, 128, 2, R), mdt)
        for j in range(GPC):
            g = d * GPC + j
            c = int(counts[g])
            if c:
                seg = x[offs[g] : offs[g] + c]  # [c, 256]
                xp[j, :, :, :c] = seg.T.reshape(2, 128, c).transpose(1, 0, 2)
        W = weight[d * GPC : (d + 1) * GPC]  # [GPC, OUT, IN]
        wt = W.reshape(GPC, OUT_F, 2, 128).transpose(3, 0, 2, 1)  # [128,GPC,2,OUT]
        bt = bias[d * GPC : (d + 1) * GPC, 0, :].reshape(GPC, 2, 128)
        bt = bt.transpose(2, 0, 1)  # [128, GPC, 2]
        in_maps.append(
            {
                "xp": np.ascontiguousarray(xp.reshape(GPC, 128, 2 * R)),
                "wt": np.ascontiguousarray(wt.reshape(128, GPC * 512)).astype(mdt),
                "bt": np.ascontiguousarray(bt.reshape(128, GPC * 2)),
            }
        )
    return in_maps


def _unpack_outputs(results, counts, offs, B, n):
    R = B * 128
    y = np.empty((n, OUT_F), np.float32)
    for d in range(N_CORES):
        ypd = np.asarray(results[d]["yp"]).reshape(GPC, 128, 2, R)
        for j in range(GPC):
            g = d * GPC + j
            c = int(counts[g])
            if c:
                y[offs[g] : offs[g] + c] = (
                    ypd[j, :, :, :c].transpose(1, 0, 2).reshape(256, c).T
                )
    return y


def kernel(weight, bias, x, counts):
    from concourse.bass_utils import run_bass_kernel_spmd

    weight = np.ascontiguousarray(np.asarray(weight), dtype=np.float32)
    bias = np.ascontiguousarray(np.asarray(bias), dtype=np.float32)
    x = np.ascontiguousarray(np.asarray(x), dtype=np.float32)
    counts = np.asarray(counts).astype(np.int64)
    n = x.shape[0]
    offs = np.zeros(G + 1, np.int64)
    np.cumsum(counts, out=offs[1:])
    B = max(1, -(-int(counts.max()) // 128))

    nc = _get_program(B)
    in_maps = _pack_inputs(weight, bias, x, counts, offs, B)
    res = run_bass_kernel_spmd(nc, in_maps, list(range(N_CORES)))
    return _unpack_outputs(res.results, counts, offs, B, n)


# revision 19
# speedup vs baseline: 1.6685x; 1.5814x over previous
"""DynamicMultiLinear (MoE-style grouped linear) Trainium2 kernel.

Problem: y[i] = x[i] @ W[g(i)].T + b[g(i)] where rows of x are contiguous
segments per network g (G=256 networks, IN=OUT=256, N=262144 rows).

Strategy (expert-parallel, per sharding hint):
  - Shard the group axis over 8 cores: core d owns networks [32d, 32d+32)
    and their contiguous row segments of x.
  - Host packs, per core, a transposed + 128-padded activation tensor
    xp[j, p, k, r] = x_seg[r, k*128+p] so the device only does dense,
    statically-shaped work (same program on all 8 cores; pad count B =
    ceil(max(counts)/128) blocks per network is a compile-time constant).
  - Device: for each network j, yT[m-chunk] = sum_k W^T[k,m-chunk].T @ xT[k]
    via fp32 matmuls accumulated in PSUM, bias fused into the PSUM->SBUF
    copy on the scalar engine.
  - Host unpads/transposes back to y[N, 256].
"""

import sys
from contextlib import ExitStack

import numpy as np

if "/opt/trn_rl_repo" not in sys.path:
    sys.path.insert(0, "/opt/trn_rl_repo")

G = 256
IN_F = 256
OUT_F = 256
N_CORES = 8
GPC = G // N_CORES  # networks per core
MODE = "fp32"  # "fp32" (exact) or "bf16" (faster, ~1e-3 error)


def _row_groups(B):
    """Split B 128-row blocks into matmul row-group sizes <= 512 (fp32 moving
    operand limit), as few groups as possible."""
    groups = []
    rem = B
    while rem > 0:
        take = min(rem, 4)
        groups.append(take * 128)
        rem -= take
    return groups


def _split_multi_waits(bir):
    """The walrus build here supports only ONE sem-wait per instruction
    (setupSyncWait: 'Too many sync wait commands'). Hoist extra waits onto
    same-engine NoOps inserted directly before the instruction — engines
    dispatch their stream in order, so gating semantics are preserved."""
    changed = False
    for fn in bir["functions"]:
        for bb in fn["blocks"]:
            out = []
            for ins in bb["instructions"]:
                si = ins.get("sync_info")
                waits = (si or {}).get("on_wait") or []
                if len(waits) > 1:
                    changed = True
                    for i, w in enumerate(waits[:-1]):
                        out.append(
                            {
                                "debug": ins.get("debug", 0),
                                "engine": ins["engine"],
                                "ins": [],
                                "name": f"{ins['name']}-sw{i}",
                                "opcode": "NoOp",
                                "outs": [],
                                "sync_info": {"on_update": [], "on_wait": [w]},
                                "text_hint": "split_wait",
                            }
                        )
                    si["on_wait"] = [waits[-1]]
                out.append(ins)
            bb["instructions"] = out
    return changed


def _patch_to_json():
    import json

    import concourse.bass as bass

    if getattr(bass.Bass, "_split_waits_patched", False):
        return
    orig = bass.Bass.to_json_bytes

    def to_json_bytes(self):
        data = orig(self)
        bir = json.loads(data)
        if _split_multi_waits(bir):
            data = json.dumps(bir).encode()
        return data

    bass.Bass.to_json_bytes = to_json_bytes
    bass.Bass._split_waits_patched = True


def _patch_tile_drain():
    """The walrus build in this container rejects sem waits on InstDrain
    ("Too many sync wait commands", CoreV3GenImpl setupSyncWait). Re-emit the
    TileContext exit drain's waits as NOPs (which do accept waits) ahead of a
    wait-free drain — same sequencer, same semantics."""
    import bass_rust
    from concourse import tile
    from concourse.tile import ScopedClock

    if getattr(tile.TileContext, "_drain_patched", False):
        return

    def _drain_and_barrier(self, tick_clock, wait_clock):
        gc = tick_clock.global_clock
        procs = [i for i in range(27) if gc[i] > 0]
        maxw = 1
        for i0 in range(0, len(procs), maxw):
            nop = self.nc.sync.nop(nofuse=True, hint="predrain_wait")
            masked = bass_rust.VectorClock()
            for i in procs[i0 : i0 + maxw]:
                masked.require_at_least(i, gc[i])
            wait_clock.add_sem_waits(nop.ins, ScopedClock({None: masked}))
        self.nc.sync.drain()
        self.nc.all_engine_barrier()
        popped = self.nc._tile_sem_poison_stack.pop()
        assert popped is self._sem_poison
        self.nc.clear_and_free_semaphores(list(self.sems.allocated().values()))
        self.nc.all_engine_barrier()

    tile.TileContext._drain_and_barrier = _drain_and_barrier
    tile.TileContext._drain_patched = True


def _build_program(B, gpc=GPC, mode=None):
    import concourse.bass as bass
    import concourse.mybir as mybir
    from concourse import tile

    _patch_tile_drain()
    _patch_to_json()

    mode = mode or MODE
    R = B * 128
    f32 = mybir.dt.float32
    mdt = mybir.dt.bfloat16 if mode == "bf16" else f32

    nc = bass.Bass(target_bir_lowering=False)
    xp = nc.declare_dram_parameter("xp", [gpc, 128, 2 * R], mdt, isOutput=False)
    wt = nc.declare_dram_parameter("wt", [128, gpc * 512], mdt, isOutput=False)
    bt = nc.declare_dram_parameter("bt", [128, gpc * 2], f32, isOutput=False)
    yp = nc.declare_dram_parameter("yp", [gpc, 128, 2 * R], f32, isOutput=True)

    rgs = _row_groups(B)
    with ExitStack() as ctx:
        tc = ctx.enter_context(tile.TileContext(nc))
        wpool = ctx.enter_context(tc.tile_pool(name="w", bufs=1))
        bpool = ctx.enter_context(tc.tile_pool(name="b", bufs=1))
        xpool = ctx.enter_context(tc.tile_pool(name="x", bufs=6))
        ypool = ctx.enter_context(tc.tile_pool(name="y", bufs=4))
        pspool = ctx.enter_context(tc.tile_pool(name="ps", bufs=8, space="PSUM"))

        # weight quads (~1MB DMAs) interleaved into the network loop so the
        # first matmuls are gated only on the first quad + first x tile
        quad = 4
        while gpc % quad:
            quad //= 2
        w_quads = [None] * (gpc // quad)

        def load_quad(q):
            w_q = wpool.tile([128, quad * 512], mdt, tag=f"w{q}", name=f"w{q}")
            nc.sync.dma_start(w_q[:], wt[:, q * quad * 512 : (q + 1) * quad * 512])
            w_quads[q] = w_q

        b_all = bpool.tile([128, gpc * 2], f32)

        yp4 = yp.rearrange("j p (m r) -> j p m r", m=2)
        for j in range(gpc):
            x_t = xpool.tile([128, 2 * R], mdt)
            nc.sync.dma_start(x_t[:], xp[j])
            if j % quad == 0:
                load_quad(j // quad)
            if j == 0:
                nc.sync.dma_start(b_all[:], bt[:])
            w_j = w_quads[j // quad][:, (j % quad) * 512 : (j % quad + 1) * 512]
            y_t = ypool.tile([128, 2 * R], f32)
            for m in range(2):
                pss = [
                    pspool.tile([128, rg], f32, tag="ps", name="ps") for rg in rgs
                ]
                # k outer, rg inner: consecutive matmuls share the stationary
                # operand (weight chunk) so PE weight reloads pipeline
                for k in range(2):
                    w0 = k * 256 + m * 128
                    r0 = 0
                    for ps, rg in zip(pss, rgs):
                        nc.tensor.matmul(
                            ps[:],
                            w_j[:, w0 : w0 + 128],
                            x_t[:, k * R + r0 : k * R + r0 + rg],
                            start=(k == 0),
                            stop=(k == 1),
                        )
                        r0 += rg
                r0 = 0
                for ps, rg in zip(pss, rgs):
                    nc.scalar.activation(
                        y_t[:, m * R + r0 : m * R + r0 + rg],
                        ps[:],
                        mybir.ActivationFunctionType.Identity,
                        bias=b_all[:, 2 * j + m : 2 * j + m + 1],
                    )
                    r0 += rg
                # per-m output DMA so the kernel tail is one half-tile deep
                nc.sync.dma_start(yp4[j, :, m, :], y_t[:, m * R : (m + 1) * R])
    return nc


_PROGRAM_CACHE = {}


def _get_program(B):
    key = (B, MODE)
    if key not in _PROGRAM_CACHE:
        _PROGRAM_CACHE[key] = _build_program(B)
    return _PROGRAM_CACHE[key]


def _pack_inputs(weight, bias, x, counts, offs, B):
    """Build the 8 per-core input maps (transpose + pad on host)."""
    if MODE == "bf16":
        import ml_dtypes

        mdt = np.dtype(ml_dtypes.bfloat16)
    else:
        mdt = np.dtype(np.float32)
    R = B * 128
    in_maps = []
    for d in range(N_CORES):
        xp = np.zeros((GPC, 128, 2, R), mdt)
        for j in range(GPC):
            g = d * GPC + j
            c = int(counts[g])
            if c:
                seg = x[offs[g] : offs[g] + c]  # [c, 256]
                xp[j, :, :, :c] = seg.T.reshape(2, 128, c).transpose(1, 0, 2)
        W = weight[d * GPC : (d + 1) * GPC]  # [GPC, OUT, IN]
        wt = W.reshape(GPC, OUT_F, 2, 128).transpose(3, 0, 2, 1)  # [128,GPC,2,OUT]
        bt = bias[d * GPC : (d + 1) * GPC, 0, :].reshape(GPC, 2, 128)
        bt = bt.transpose(2, 0, 1)  # [128, GPC, 2]
        in_maps.append(
            {
                "xp": np.ascontiguousarray(xp.reshape(GPC, 128, 2 * R)),
                "wt": np.ascontiguousarray(wt.reshape(128, GPC * 512)).astype(mdt),
                "bt": np.ascontiguousarray(bt.reshape(128, GPC * 2)),
            }
        )
    return in_maps


def _unpack_outputs(results, counts, offs, B, n):
    R = B * 128
    y = np.empty((n, OUT_F), np.float32)
    for d in range(N_CORES):
        ypd = np.asarray(results[d]["yp"]).reshape(GPC, 128, 2, R)
        for j in range(GPC):
            g = d * GPC + j
            c = int(counts[g])
            if c:
                y[offs[g] : offs[g] + c] = (
                    ypd[j, :, :, :c].transpose(1, 0, 2).reshape(256, c).T
                )
    return y


def kernel(weight, bias, x, counts):
    from concourse.bass_utils import run_bass_kernel_spmd

    weight = np.ascontiguousarray(np.asarray(weight), dtype=np.float32)
    bias = np.ascontiguousarray(np.asarray(bias), dtype=np.float32)
    x = np.ascontiguousarray(np.asarray(x), dtype=np.float32)
    counts = np.asarray(counts).astype(np.int64)
    n = x.shape[0]
    offs = np.zeros(G + 1, np.int64)
    np.cumsum(counts, out=offs[1:])
    B = max(1, -(-int(counts.max()) // 128))

    nc = _get_program(B)
    in_maps = _pack_inputs(weight, bias, x, counts, offs, B)
    res = run_bass_kernel_spmd(nc, in_maps, list(range(N_CORES)))
    return _unpack_outputs(res.results, counts, offs, B, n)
